# revision 10
# baseline (speedup 1.0000x reference)
"""Causal self-attention Trainium2 kernel (8 NeuronCores).

Sharding: data-parallel over batch (2) x tensor-parallel over head groups
(12 heads -> 4 groups of 3). Core c handles batch c//4, head group c%4.
Each core computes its partial projection output; the host sums the 4
partials per batch (the TP all-reduce done on host, since the output
gather happens anyway).

Per-core dataflow (T=2048, C=768, local heads h0..h2, HD=64):
  x [T,C] --PE transpose--> xT [C,T]
  qkvT [576,T] = Wqkv_local.T @ x.T   (Wqkv columns permuted, see below)
  per head: S^T_j [tk=128, tq] = k_h^T.T-slice @ q_h^T  (K=64 matmuls)
            P = exp(S^T/8), causal diag masked via triangular 0/1 mul
            y^T [65, tq] += [v_h | ones].T @ P_j  (row 64 = softmax denom)
            y_h^T = y^T[0:64] * bcast(1/denom)
  out_partial [T, C] = y^T.T-slices @ Wproj_local, DMA to HBM.

Wqkv local column order (64-col blocks): [q0 q1 k0 k1 q2 v0 k2 v1 v2]
so that q_h and k_h land at matching SBUF partition offsets (0 or 64)
for K=64 matmuls, and heads 0/1 can row-pack the PE array.
"""

import functools
import os

import numpy as np

import concourse.bass as bass
import concourse.mybir as mybir
import concourse.tile as tile
from concourse import bacc
from concourse.bass_utils import run_bass_kernel_spmd
from concourse.masks import make_identity, make_upper_triangular

P = 128
B, T, C = 2, 2048, 768
NH, HD = 12, 64
HPG = 3          # heads per core
G = 4            # head groups
LCH = HPG * HD   # 192 local channels
QKV_CH = 3 * LCH  # 576
NT = T // P      # 16 t-tiles
NCC = C // P     # 6 contraction tiles
F32 = mybir.dt.float32
F32R = mybir.dt.float32r

LAST_RESULT = None
NPHASE = float(os.environ.get("KERNEL_NPHASE", "4"))


def _emit(nc, tc, x_d, wqkv_d, wproj_d, out_d):
    from contextlib import ExitStack

    ctx = ExitStack()
    with ctx:
        const = ctx.enter_context(tc.tile_pool(name="const", bufs=1))
        ident_f32 = const.tile([P, P], F32)
        make_identity(nc, ident_f32[:])
        ident = const.tile([P, P], F32R)
        nc.any.tensor_copy(out=ident[:], in_=ident_f32[:])
        tri = const.tile([P, P], F32)
        make_upper_triangular(nc, tri[:], val=1.0, diag=True)
        ones_f32 = const.tile([P, HD], F32)
        nc.any.memset(ones_f32[:], 1.0)
        ones64 = const.tile([P, HD], F32R)
        nc.any.tensor_copy(out=ones64[:], in_=ones_f32[:])

        wq_pool = ctx.enter_context(tc.tile_pool(name="wq", bufs=1))
        wqkv_sb = []
        for cc in range(NCC):
            t = wq_pool.tile([P, QKV_CH], F32R, tag=f"wq{cc}")
            nc.sync.dma_start(t[:], wqkv_d[cc * P : (cc + 1) * P, :])
            wqkv_sb.append(t)

        wp_pool = ctx.enter_context(tc.tile_pool(name="wp", bufs=1))
        wp_h = []
        for hh in range(HPG):
            wpt = wp_pool.tile([HD, C], F32R, tag=f"wp{hh}", name=f"wp{hh}")
            nc.sync.dma_start(wpt[:], wproj_d[HD * hh : HD * (hh + 1), :])
            wp_h.append(wpt)

        big_pool = ctx.enter_context(tc.tile_pool(name="big", bufs=1))
        xs_pool = ctx.enter_context(tc.tile_pool(name="xs", bufs=3))
        qkvt_pool = ctx.enter_context(tc.tile_pool(name="qkvt", bufs=1))
        v_pool = ctx.enter_context(tc.tile_pool(name="v", bufs=1))
        y_pool = ctx.enter_context(tc.tile_pool(name="y", bufs=1))
        nrm_pool = ctx.enter_context(tc.tile_pool(name="nrm", bufs=2))
        out_pool = ctx.enter_context(tc.tile_pool(name="outp", bufs=3))

        # ---------------- phase 1: x -> xT (PE transposes) ----------------
        xT = big_pool.tile([P, NCC * T], F32R, tag="big")  # xT[:, 2048*cc + t]
        with tc.tile_pool(name="ps_xp", bufs=3, space="PSUM") as ps_xp:
            for tt in range(NT):
                xt = xs_pool.tile([P, C], F32R, tag="x")
                nc.sync.dma_start(xt[:], x_d[tt * P : (tt + 1) * P, :])
                for grp, ncc in ((0, 4), (4, 2)):
                    pst = ps_xp.tile([P, ncc * P], F32R, tag="xp")
                    for k in range(ncc):
                        cc = grp + k
                        nc.tensor.transpose(
                            pst[:, k * P : (k + 1) * P],
                            xt[:, cc * P : (cc + 1) * P],
                            ident[:],
                        )
                    # strided copy: psum [128, ncc*128] -> xT columns per cc
                    dst = xT[:].rearrange("p (c t) -> p c t", c=NCC)[
                        :, grp : grp + ncc, tt * P : (tt + 1) * P
                    ]
                    src = pst[:].rearrange("p (c t) -> p c t", c=ncc)
                    nc.any.tensor_copy(out=dst, in_=src)

        if NPHASE < 2:
            return
        # ---------------- phase 2: qkvT = Wqkv_local.T @ xT ----------------
        # qkvT partition-tiles: [q0|q1], [k0|k1], [q2|v0], [k2|v1], [v2]
        ch_tiles = [(0, P), (P, P), (2 * P, P), (3 * P, P), (4 * P, HD)]
        qkvT = []
        for i, (ch0, chw) in enumerate(ch_tiles):
            qkvT.append(
                qkvt_pool.tile([chw, T], F32R, tag=f"qkvt{i}", name=f"qkvT{i}")
            )
        with tc.tile_pool(name="ps_qkv", bufs=4, space="PSUM") as ps_qkv:
            for i, (ch0, chw) in enumerate(ch_tiles):
                for tch in range(T // 512):
                    ps = ps_qkv.tile([chw, 512], F32, tag="qkv")
                    for cc in range(NCC):
                        nc.tensor.matmul(
                            ps[:],
                            wqkv_sb[cc][:, ch0 : ch0 + chw],
                            xT[:, cc * T + tch * 512 : cc * T + (tch + 1) * 512],
                            start=(cc == 0),
                            stop=(cc == NCC - 1),
                        )
                    nc.any.tensor_copy(
                        out=qkvT[i][:, tch * 512 : (tch + 1) * 512], in_=ps[:]
                    )

        # head slices (tile index, partition offset)
        q_sl = [(0, 0), (0, HD), (2, 0)]
        k_sl = [(1, 0), (1, HD), (3, 0)]
        v_sl = [(2, HD), (3, HD), (4, 0)]

        if NPHASE < 2.5:
            return
        # ---------------- phase 2.5: v^T -> v (+ ones col) ----------------
        # v_sb[h]: [128, 16*65]; col 65*jt+64 is the ones column
        v_sb = []
        with tc.tile_pool(name="ps_vt", bufs=3, space="PSUM") as ps_vt:
            for h in range(HPG):
                vt = v_pool.tile([P, NT * (HD + 1)], F32R, tag=f"v{h}")
                ones_cols = vt[:].rearrange("p (t d) -> p t d", d=HD + 1)[:, :, HD:]
                src_ones = ones_f32[:].rearrange("p (a b) -> p a b", b=1)[:, 0:NT, :]
                nc.any.tensor_copy(out=ones_cols, in_=src_ones)
                ti, po = v_sl[h]
                vTh = qkvT[ti][po : po + HD, :]
                idn = ident[po : po + HD, po : po + HD]
                for half in range(2):  # 8 t-tiles per psum bank
                    pst = ps_vt.tile([P, 8 * HD], F32R, tag="vt")
                    for k in range(8):
                        jt = half * 8 + k
                        nc.tensor.transpose(
                            pst[:, k * HD : (k + 1) * HD],
                            vTh[:, jt * P : (jt + 1) * P],
                            idn,
                        )
                    dst = vt[:].rearrange("p (t d) -> p t d", d=HD + 1)[
                        :, half * 8 : half * 8 + 8, 0:HD
                    ]
                    src = pst[:].rearrange("p (t d) -> p t d", d=HD)
                    nc.any.tensor_copy(out=dst, in_=src)
                v_sb.append(vt)

        if NPHASE < 3:
            return
        # ---------------- phase 3: attention per head ----------------
        # exp buffer: row j at free offset off_j, width 2048-128j
        offs = []
        o = 0
        for j in range(NT):
            offs.append(o)
            o += T - P * j
        exp_sb = big_pool.tile([P, o], F32R, tag="big")

        yT_h = [
            y_pool.tile([HD, T], F32R, tag=f"y{hh}", name=f"yT{hh}")
            for hh in range(HPG)
        ]

        with tc.tile_pool(name="ps_att", bufs=1, space="PSUM") as ps_att:
            for h in range(HPG):
                qi, qo = q_sl[h]
                ki, ko = k_sl[h]
                qh = qkvT[qi][qo : qo + HD, :]
                kh = qkvT[ki][ko : ko + HD, :]

                # QK^T (S^T layout) + exp
                for j in range(NT):
                    w = T - P * j
                    tq0 = P * j
                    done = 0
                    while done < w:
                        cw = min(1024, w - done)
                        st = ps_att.tile([P, 1024], F32, tag="st", bufs=2)
                        for s0 in range(0, cw, 512):
                            sw = min(512, cw - s0)
                            nc.tensor.matmul(
                                st[:, s0 : s0 + sw],
                                kh[:, tq0 : tq0 + P],
                                qh[:, tq0 + done + s0 : tq0 + done + s0 + sw],
                                start=True,
                                stop=True,
                            )
                        nc.scalar.activation(
                            exp_sb[:, offs[j] + done : offs[j] + done + cw],
                            st[:, 0:cw],
                            mybir.ActivationFunctionType.Exp,
                            scale=0.125,
                        )
                        done += cw
                    # causal mask on the diagonal 128-block
                    dg = exp_sb[:, offs[j] : offs[j] + P]
                    nc.any.tensor_mul(out=dg, in0=dg, in1=tri[:])

                # PV: y^T[65, T] accumulated over j
                yp = ps_att.tile([HD + 1, T], F32, tag="y")
                for j in range(NT):
                    va = v_sb[h][:, j * (HD + 1) : (j + 1) * (HD + 1)]
                    q0 = j // 4
                    for q in range(q0, 4):
                        lo = max(512 * q, P * j)
                        hi = 512 * (q + 1)
                        nc.tensor.matmul(
                            yp[:, lo:hi],
                            va,
                            exp_sb[:, offs[j] + lo - P * j : offs[j] + hi - P * j],
                            start=(j == 0),
                            stop=(j == 4 * q + 3),
                        )

                if NPHASE < 3.5:
                    continue
                # normalize: y_h^T = yp[0:64] * bcast(1/denom)
                rcp = nrm_pool.tile([P, T], F32R, tag="rcp", bufs=1)
                with nc.allow_low_precision(reason="fp32r softmax denom"):
                    nc.vector.reciprocal(rcp[HD : HD + 1, :], yp[HD : HD + 1, :])
                ydst = yT_h[h][:, :]
                for q in range(4):
                    bc = ps_att.tile([HD, 512], F32, tag="st", bufs=2)
                    nc.tensor.matmul(
                        bc[:],
                        ones64[HD : HD + 1, :],
                        rcp[HD : HD + 1, 512 * q : 512 * (q + 1)],
                        start=True,
                        stop=True,
                    )
                    bcs = nrm_pool.tile([HD, 512], F32, tag="bcs")
                    nc.any.tensor_copy(out=bcs[:], in_=bc[:])
                    nc.vector.tensor_mul(
                        out=ydst[:, 512 * q : 512 * (q + 1)],
                        in0=yp[0:HD, 512 * q : 512 * (q + 1)],
                        in1=bcs[:],
                    )

        if NPHASE < 4:
            return
        # ---------------- phase 4: proj ----------------
        with tc.tile_pool(name="ps_prj", bufs=3, space="PSUM") as ps_prj:
            for tt in range(NT):
                pj = ps_prj.tile([P, C], F32, tag="pj")
                lhs_w = [
                    (yT_h[hh][:, tt * P : (tt + 1) * P], wp_h[hh][:, :])
                    for hh in range(HPG)
                ]
                for ki_, (lhs, wrow) in enumerate(lhs_w):
                    for n0, nw in ((0, 512), (512, 256)):
                        nc.tensor.matmul(
                            pj[:, n0 : n0 + nw],
                            lhs,
                            wrow[:, n0 : n0 + nw],
                            start=(ki_ == 0),
                            stop=(ki_ == 2),
                        )
                ot = out_pool.tile([P, C], F32, tag="o")
                nc.any.tensor_copy(out=ot[:], in_=pj[:])
                nc.sync.dma_start(out_d[tt * P : (tt + 1) * P, :], ot[:])


@functools.cache
def _build():
    nc = bacc.Bacc(
        "TRN2",
        target_bir_lowering=False,
        debug=False,
        enable_asserts=False,
        num_devices=8,
    )
    x_d = nc.dram_tensor("x", [T, C], F32R, kind="ExternalInput").ap()
    wqkv_d = nc.dram_tensor("wqkv", [C, QKV_CH], F32R, kind="ExternalInput").ap()
    wproj_d = nc.dram_tensor("wproj", [LCH, C], F32R, kind="ExternalInput").ap()
    out_d = nc.dram_tensor("out", [T, C], F32, kind="ExternalOutput").ap()
    with tile.TileContext(nc) as tc:
        _emit(nc, tc, x_d, wqkv_d, wproj_d, out_d)
    nc.compile()
    return nc


def kernel(x, mask, Wqkv, Wproj):
    global LAST_RESULT
    x = np.ascontiguousarray(np.asarray(x, dtype=np.float32))
    Wqkv = np.asarray(Wqkv, dtype=np.float32)
    Wproj = np.asarray(Wproj, dtype=np.float32)

    in_maps = []
    for c in range(8):
        b, g = divmod(c, 4)
        hs = [3 * g, 3 * g + 1, 3 * g + 2]  # global heads

        def qcol(h):
            return Wqkv[:, 64 * h : 64 * h + 64]

        def kcol(h):
            return Wqkv[:, C + 64 * h : C + 64 * h + 64]

        def vcol(h):
            return Wqkv[:, 2 * C + 64 * h : 2 * C + 64 * h + 64]

        wq = np.concatenate(
            [
                qcol(hs[0]), qcol(hs[1]),
                kcol(hs[0]), kcol(hs[1]),
                qcol(hs[2]), vcol(hs[0]),
                kcol(hs[2]), vcol(hs[1]),
                vcol(hs[2]),
            ],
            axis=1,
        )
        wp = Wproj[LCH * g : LCH * (g + 1), :]
        in_maps.append(
            {
                "x": np.ascontiguousarray(x[b]),
                "wqkv": np.ascontiguousarray(wq),
                "wproj": np.ascontiguousarray(wp),
            }
        )

    nc = _build()
    res = run_bass_kernel_spmd(nc, in_maps, core_ids=list(range(8)))
    LAST_RESULT = res
    out = np.empty((B, T, C), dtype=np.float32)
    for b in range(B):
        acc = res.results[4 * b]["out"].astype(np.float32)
        for g in range(1, 4):
            acc = acc + res.results[4 * b + g]["out"]
        out[b] = acc
    return out


if __name__ == "__main__":
    rng = np.random.default_rng(0)
    x = rng.standard_normal((B, T, C), dtype=np.float32)
    wqkv = rng.standard_normal((C, 3 * C), dtype=np.float32) / np.sqrt(C)
    wproj = rng.standard_normal((C, C), dtype=np.float32) / np.sqrt(C)
    o = kernel(x, None, wqkv, wproj)
    print(o.shape, o.dtype)


# revision 11
# speedup vs baseline: 1.1751x; 1.1751x over previous
"""Causal self-attention Trainium2 kernel (8 NeuronCores).

Sharding: data-parallel over batch (2) x tensor-parallel over head groups
(12 heads -> 4 groups of 3). Core c handles batch c//4, head group c%4.
Each core computes its partial projection output; the host sums the 4
partials per batch (the TP reduce folded into the output gather).

Per-core dataflow (T=2048, C=768, local heads h0..h2, HD=64):
  x [T,C] --PE transpose--> xT [C,T]            (fp32r)
  qkvT [576,T] = Wqkv_local.T @ x.T             (fp32r matmuls -> bf16 out)
  per head: S^T_j [tk=128, tq] = k_h slice.T @ q_h^T  (bf16, K=64,
            heads 0/1 row-packed in the PE array via partition offsets)
            P = exp(S^T/8) (ACT, bf16 out), causal diag masked on GpSimd
            y^T chunk [65, 512] += [v_h | ones].T @ P_j (row 64 = denom)
            y_h^T = y^T[0:64] * bcast(1/denom)  (PE bcast + DVE mul)
  out_partial [T, C] = y^T.T-slices @ Wproj_local (bf16), DMA to HBM.

Wqkv local column order (64-col blocks): [q0 q1 k0 k1 q2 v0 k2 v1 v2]
so q_h/k_h of heads 0,1 land at partition offsets 0/64 -> K=64 QK^T
matmuls of the two heads occupy disjoint PE row groups and overlap.
"""

import functools
import os

import numpy as np

import concourse.bass as bass
import concourse.mybir as mybir
import concourse.tile as tile
from concourse import bacc
from concourse.bass_utils import run_bass_kernel_spmd
from concourse.masks import make_identity, make_upper_triangular

P = 128
B, T, C = 2, 2048, 768
NH, HD = 12, 64
HPG = 3           # heads per core
LCH = HPG * HD    # 192 local channels
QKV_CH = 3 * LCH  # 576
NT = T // P       # 16 t-tiles
NCC = C // P      # 6 contraction tiles
NQ = T // 512     # 4 query chunks
F32 = mybir.dt.float32
F32R = mybir.dt.float32r
BF16 = mybir.dt.bfloat16

# causal exp-buffer layout: row j at offset OFFS[j], width 2048-128*j
OFFS = []
_o = 0
for _j in range(NT):
    OFFS.append(_o)
    _o += T - P * _j
EXPW = _o  # 17408

LAST_RESULT = None


def _emit(nc, tc, x_d, wqkv_d, wproj_d, out_d):
    from contextlib import ExitStack

    ctx = ExitStack()
    with ctx:
        const = ctx.enter_context(tc.tile_pool(name="const", bufs=1))
        ident_f32 = const.tile([P, P], F32)
        make_identity(nc, ident_f32[:])
        ident = const.tile([P, P], F32R)
        nc.vector.tensor_copy(out=ident[:], in_=ident_f32[:])
        identb = const.tile([P, P], BF16)
        nc.vector.tensor_copy(out=identb[:], in_=ident_f32[:])
        tri = const.tile([P, P], BF16)
        make_upper_triangular(nc, tri[:], val=1.0, diag=True)
        ones_f32 = const.tile([P, HD], F32)
        nc.any.memset(ones_f32[:], 1.0)
        ones64 = const.tile([P, HD], F32R)
        nc.vector.tensor_copy(out=ones64[:], in_=ones_f32[:])

        wq_pool = ctx.enter_context(tc.tile_pool(name="wq", bufs=1))
        wqkv_sb = []
        for cc in range(NCC):
            t = wq_pool.tile([P, QKV_CH], F32R, tag=f"wq{cc}")
            nc.sync.dma_start(t[:], wqkv_d[cc * P : (cc + 1) * P, :])
            wqkv_sb.append(t)

        wp_pool = ctx.enter_context(tc.tile_pool(name="wp", bufs=1))
        wpf_a = wp_pool.tile([P, C], F32, tag="wpfa")
        nc.sync.dma_start(wpf_a[:], wproj_d[0:P, :])
        wpf_b = wp_pool.tile([HD, C], F32, tag="wpfb")
        nc.sync.dma_start(wpf_b[:], wproj_d[P : P + HD, :])
        wp_a = wp_pool.tile([P, C], BF16, tag="wpa")
        nc.vector.tensor_copy(out=wp_a[:], in_=wpf_a[:])
        wp_b = wp_pool.tile([HD, C], BF16, tag="wpb")
        nc.vector.tensor_copy(out=wp_b[:], in_=wpf_b[:])

        big_pool = ctx.enter_context(tc.tile_pool(name="big", bufs=1))
        xs_pool = ctx.enter_context(tc.tile_pool(name="xs", bufs=3))
        qkvt_pool = ctx.enter_context(tc.tile_pool(name="qkvt", bufs=1))
        v_pool = ctx.enter_context(tc.tile_pool(name="v", bufs=1))
        y_pool = ctx.enter_context(tc.tile_pool(name="y", bufs=1))
        nrm_pool = ctx.enter_context(tc.tile_pool(name="nrm", bufs=2))
        out_pool = ctx.enter_context(tc.tile_pool(name="outp", bufs=3))

        # ---------------- phase 1: x -> xT (PE transposes, fp32r) ----------
        xT = big_pool.tile([P, NCC * T], F32R, tag="big")  # xT[:, 2048*cc + t]
        with tc.tile_pool(name="ps_xp", bufs=3, space="PSUM") as ps_xp:
            for tt in range(NT):
                xt = xs_pool.tile([P, C], F32R, tag="x")
                nc.sync.dma_start(xt[:], x_d[tt * P : (tt + 1) * P, :])
                for grp, ncc in ((0, 4), (4, 2)):
                    pst = ps_xp.tile([P, ncc * P], F32R, tag="xp")
                    for k in range(ncc):
                        cc = grp + k
                        nc.tensor.transpose(
                            pst[:, k * P : (k + 1) * P],
                            xt[:, cc * P : (cc + 1) * P],
                            ident[:],
                        )
                    dst = xT[:].rearrange("p (c t) -> p c t", c=NCC)[
                        :, grp : grp + ncc, tt * P : (tt + 1) * P
                    ]
                    src = pst[:].rearrange("p (c t) -> p c t", c=ncc)
                    nc.vector.tensor_copy(out=dst, in_=src)

        # ---------------- phase 2: qkvT = Wqkv_local.T @ xT (fp32r) --------
        # qkvT partition-tiles: [q0|q1], [k0|k1], [q2|v0], [k2|v1], [v2]
        ch_tiles = [(0, P), (P, P), (2 * P, P), (3 * P, P), (4 * P, HD)]
        qkvT = []
        for i, (ch0, chw) in enumerate(ch_tiles):
            qkvT.append(
                qkvt_pool.tile([chw, T], BF16, tag=f"qkvt{i}", name=f"qkvT{i}")
            )
        with tc.tile_pool(name="ps_qkv", bufs=4, space="PSUM") as ps_qkv:
            for i, (ch0, chw) in enumerate(ch_tiles):
                for tch in range(NQ):
                    ps = ps_qkv.tile([chw, 512], F32, tag="qkv")
                    for cc in range(NCC):
                        nc.tensor.matmul(
                            ps[:],
                            wqkv_sb[cc][:, ch0 : ch0 + chw],
                            xT[:, cc * T + tch * 512 : cc * T + (tch + 1) * 512],
                            start=(cc == 0),
                            stop=(cc == NCC - 1),
                        )
                    nc.vector.tensor_copy(
                        out=qkvT[i][:, tch * 512 : (tch + 1) * 512], in_=ps[:]
                    )

        # head slices (tile index, partition offset)
        q_sl = [(0, 0), (0, HD), (2, 0)]
        k_sl = [(1, 0), (1, HD), (3, 0)]
        v_sl = [(2, HD), (3, HD), (4, 0)]

        # ---------------- phase 2.5: v^T -> v (+ ones col), bf16 -----------
        # v_sb[h]: [128, 16*65]; col 65*jt+64 is the ones column
        v_sb = []
        with tc.tile_pool(name="ps_vt", bufs=3, space="PSUM") as ps_vt:
            for h in range(HPG):
                vt = v_pool.tile([P, NT * (HD + 1)], BF16, tag=f"v{h}")
                ones_cols = vt[:].rearrange("p (t d) -> p t d", d=HD + 1)[:, :, HD:]
                src_ones = ones_f32[:].rearrange("p (a b) -> p a b", b=1)[:, 0:NT, :]
                nc.vector.tensor_copy(out=ones_cols, in_=src_ones)
                ti, po = v_sl[h]
                vTh = qkvT[ti][po : po + HD, :]
                idnb = identb[po : po + HD, po : po + HD]
                for half in range(2):  # 8 t-tiles per psum tile
                    pst = ps_vt.tile([P, 8 * HD], BF16, tag="vt")
                    for k in range(8):
                        jt = half * 8 + k
                        nc.tensor.transpose(
                            pst[:, k * HD : (k + 1) * HD],
                            vTh[:, jt * P : (jt + 1) * P],
                            idnb,
                        )
                    dst = vt[:].rearrange("p (t d) -> p t d", d=HD + 1)[
                        :, half * 8 : half * 8 + 8, 0:HD
                    ]
                    src = pst[:].rearrange("p (t d) -> p t d", d=HD)
                    nc.vector.tensor_copy(out=dst, in_=src)
                v_sb.append(vt)

        # ---------------- phase 3: attention ----------------
        # exp buffer: [128, 2*EXPW] bf16; slot 0 = first head of the pair,
        # slot 1 = second. h2 reuses slot 0.
        exp_sb = big_pool.tile([P, 2 * EXPW], BF16, tag="big")
        yT_a = y_pool.tile([P, T], BF16, tag="ya")   # h0 rows 0:64, h1 64:128
        yT_b = y_pool.tile([HD, T], BF16, tag="yb")  # h2

        def ydst_of(h):
            return yT_a[0:HD, :] if h == 0 else (
                yT_a[HD:P, :] if h == 1 else yT_b[0:HD, :]
            )

        with tc.tile_pool(name="ps_att", bufs=1, space="PSUM") as ps_att:
            for pair in ((0, 1), (2,)):
                for j in range(NT):
                    w = T - P * j
                    tq0 = P * j
                    for sl, h in enumerate(pair):
                        qi, qo = q_sl[h]
                        ki, ko = k_sl[h]
                        qh = qkvT[qi][qo : qo + HD, :]
                        kh = qkvT[ki][ko : ko + HD, :]
                        eoff = sl * EXPW + OFFS[j]
                        done = 0
                        while done < w:
                            cw = min(1024, w - done)
                            st = ps_att.tile([P, 1024], F32, tag="st", bufs=2)
                            for s0 in range(0, cw, 512):
                                sw = min(512, cw - s0)
                                nc.tensor.matmul(
                                    st[:, s0 : s0 + sw],
                                    kh[:, tq0 : tq0 + P],
                                    qh[:, tq0 + done + s0 : tq0 + done + s0 + sw],
                                    start=True,
                                    stop=True,
                                )
                            nc.scalar.activation(
                                exp_sb[:, eoff + done : eoff + done + cw],
                                st[:, 0:cw],
                                mybir.ActivationFunctionType.Exp,
                                scale=0.125,
                            )
                            done += cw
                        # causal mask on the diagonal 128-block (GpSimd)
                        dg = exp_sb[:, eoff : eoff + P]
                        nc.gpsimd.tensor_mul(out=dg, in0=dg, in1=tri[:])

                    if j % 4 == 3:
                        q = j // 4
                        for sl, h in enumerate(pair):
                            yq = ps_att.tile([HD + 1, 512], F32, tag="y", bufs=2)
                            for jj in range(4 * q + 4):
                                va = v_sb[h][:, jj * (HD + 1) : (jj + 1) * (HD + 1)]
                                lo = max(512 * q, P * jj)
                                hi = 512 * (q + 1)
                                so = sl * EXPW + OFFS[jj] - P * jj
                                nc.tensor.matmul(
                                    yq[:, lo - 512 * q : hi - 512 * q],
                                    va,
                                    exp_sb[:, so + lo : so + hi],
                                    start=(jj == 0),
                                    stop=(jj == 4 * q + 3),
                                )
                            # normalize chunk: y = y[0:64] * bcast(1/denom)
                            rcp = nrm_pool.tile([P, 512], F32R, tag="rcp")
                            with nc.allow_low_precision(reason="softmax denom"):
                                nc.vector.reciprocal(
                                    rcp[HD : HD + 1, :], yq[HD : HD + 1, :]
                                )
                            bc = ps_att.tile([HD, 512], F32, tag="bc", bufs=2)
                            nc.tensor.matmul(
                                bc[:],
                                ones64[HD : HD + 1, :],
                                rcp[HD : HD + 1, :],
                                start=True,
                                stop=True,
                            )
                            bcs = nrm_pool.tile([HD, 512], F32, tag="bcs")
                            nc.vector.tensor_copy(out=bcs[:], in_=bc[:])
                            nc.vector.tensor_mul(
                                out=ydst_of(h)[:, 512 * q : 512 * (q + 1)],
                                in0=yq[0:HD, :],
                                in1=bcs[:],
                            )

        # ---------------- phase 4: proj (bf16, K=128 + K=64) ----------------
        with tc.tile_pool(name="ps_prj", bufs=3, space="PSUM") as ps_prj:
            for tt in range(NT):
                pj = ps_prj.tile([P, C], F32, tag="pj")
                lhs_w = [
                    (yT_a[:, tt * P : (tt + 1) * P], wp_a[:, :]),
                    (yT_b[:, tt * P : (tt + 1) * P], wp_b[:, :]),
                ]
                for ki_, (lhs, wrow) in enumerate(lhs_w):
                    for n0, nw in ((0, 512), (512, 256)):
                        nc.tensor.matmul(
                            pj[:, n0 : n0 + nw],
                            lhs,
                            wrow[:, n0 : n0 + nw],
                            start=(ki_ == 0),
                            stop=(ki_ == 1),
                        )
                ot = out_pool.tile([P, C], F32, tag="o")
                nc.vector.tensor_copy(out=ot[:], in_=pj[:])
                nc.sync.dma_start(out_d[tt * P : (tt + 1) * P, :], ot[:])


@functools.cache
def _build():
    nc = bacc.Bacc(
        "TRN2",
        target_bir_lowering=False,
        debug=False,
        enable_asserts=False,
        num_devices=8,
    )
    x_d = nc.dram_tensor("x", [T, C], F32R, kind="ExternalInput").ap()
    wqkv_d = nc.dram_tensor("wqkv", [C, QKV_CH], F32R, kind="ExternalInput").ap()
    wproj_d = nc.dram_tensor("wproj", [LCH, C], F32, kind="ExternalInput").ap()
    out_d = nc.dram_tensor("out", [T, C], F32, kind="ExternalOutput").ap()
    with tile.TileContext(nc) as tc:
        _emit(nc, tc, x_d, wqkv_d, wproj_d, out_d)
    nc.compile()
    return nc


def kernel(x, mask, Wqkv, Wproj):
    global LAST_RESULT
    x = np.ascontiguousarray(np.asarray(x, dtype=np.float32))
    Wqkv = np.asarray(Wqkv, dtype=np.float32)
    Wproj = np.asarray(Wproj, dtype=np.float32)

    in_maps = []
    for c in range(8):
        b, g = divmod(c, 4)
        hs = [3 * g, 3 * g + 1, 3 * g + 2]  # global heads

        def qcol(h):
            return Wqkv[:, 64 * h : 64 * h + 64]

        def kcol(h):
            return Wqkv[:, C + 64 * h : C + 64 * h + 64]

        def vcol(h):
            return Wqkv[:, 2 * C + 64 * h : 2 * C + 64 * h + 64]

        wq = np.concatenate(
            [
                qcol(hs[0]), qcol(hs[1]),
                kcol(hs[0]), kcol(hs[1]),
                qcol(hs[2]), vcol(hs[0]),
                kcol(hs[2]), vcol(hs[1]),
                vcol(hs[2]),
            ],
            axis=1,
        )
        wp = Wproj[LCH * g : LCH * (g + 1), :]
        in_maps.append(
            {
                "x": np.ascontiguousarray(x[b]),
                "wqkv": np.ascontiguousarray(wq),
                "wproj": np.ascontiguousarray(wp),
            }
        )

    nc = _build()
    res = run_bass_kernel_spmd(nc, in_maps, core_ids=list(range(8)))
    LAST_RESULT = res
    out = np.empty((B, T, C), dtype=np.float32)
    for b in range(B):
        acc = res.results[4 * b]["out"].astype(np.float32)
        for g in range(1, 4):
            acc = acc + res.results[4 * b + g]["out"]
        out[b] = acc
    return out


if __name__ == "__main__":
    rng = np.random.default_rng(0)
    x = rng.standard_normal((B, T, C), dtype=np.float32)
    wqkv = rng.standard_normal((C, 3 * C), dtype=np.float32) / np.sqrt(C)
    wproj = rng.standard_normal((C, C), dtype=np.float32) / np.sqrt(C)
    o = kernel(x, None, wqkv, wproj)
    print(o.shape, o.dtype)


# revision 12
# speedup vs baseline: 1.2081x; 1.0280x over previous
"""Causal self-attention Trainium2 kernel (8 NeuronCores).

Sharding: data-parallel over batch (2) x tensor-parallel over head groups
(12 heads -> 4 groups of 3). Core c handles batch c//4, head group c%4.
Each core computes its partial projection output; the host sums the 4
partials per batch (the TP reduce folded into the output gather).

Per-core dataflow (T=2048, C=768, local heads h0..h2, HD=64):
  x [T,C] --PE transpose--> xT [C,T]            (fp32r)
  qkvT [576,T] = Wqkv_local.T @ x.T             (fp32r matmuls -> bf16 out)
  per head: S^T_j [tk=128, tq] = k_h slice.T @ q_h^T  (bf16, K=64,
            heads 0/1 row-packed in the PE array via partition offsets)
            P = exp(S^T/8) (ACT, bf16 out), causal diag masked on GpSimd
            y^T chunk [65, 512] += [v_h | ones].T @ P_j (row 64 = denom)
            y_h^T = y^T[0:64] * bcast(1/denom)  (PE bcast + DVE mul)
  out_partial [T, C] = y^T.T-slices @ Wproj_local (bf16), DMA to HBM.

Wqkv local column order (64-col blocks): [q0 q1 k0 k1 q2 v0 k2 v1 v2]
so q_h/k_h of heads 0,1 land at partition offsets 0/64 -> K=64 QK^T
matmuls of the two heads occupy disjoint PE row groups and overlap.
"""

import functools
import os

import numpy as np

import concourse.bass as bass
import concourse.mybir as mybir
import concourse.tile as tile
from concourse import bacc
from concourse.bass_utils import run_bass_kernel_spmd
from concourse.masks import make_identity, make_upper_triangular

P = 128
B, T, C = 2, 2048, 768
NH, HD = 12, 64
HPG = 3           # heads per core
LCH = HPG * HD    # 192 local channels
QKV_CH = 3 * LCH  # 576
NT = T // P       # 16 t-tiles
NCC = C // P      # 6 contraction tiles
NQ = T // 512     # 4 query chunks
F32 = mybir.dt.float32
F32R = mybir.dt.float32r
BF16 = mybir.dt.bfloat16

# causal exp-buffer layout: row j at offset OFFS[j], width 2048-128*j
OFFS = []
_o = 0
for _j in range(NT):
    OFFS.append(_o)
    _o += T - P * _j
EXPW = _o  # 17408

LAST_RESULT = None


def _emit(nc, tc, x_d, wqkv_d, wproj_d, out_d):
    from contextlib import ExitStack

    ctx = ExitStack()
    with ctx:
        const = ctx.enter_context(tc.tile_pool(name="const", bufs=1))
        ident_f32 = const.tile([P, P], F32)
        make_identity(nc, ident_f32[:])
        ident = const.tile([P, P], F32R)
        nc.vector.tensor_copy(out=ident[:], in_=ident_f32[:])
        identb = const.tile([P, P], BF16)
        nc.vector.tensor_copy(out=identb[:], in_=ident_f32[:])
        tri = const.tile([P, P], BF16)
        make_upper_triangular(nc, tri[:], val=1.0, diag=True)
        ones_f32 = const.tile([P, HD], F32)
        nc.any.memset(ones_f32[:], 1.0)
        ones64 = const.tile([P, HD], F32R)
        nc.vector.tensor_copy(out=ones64[:], in_=ones_f32[:])

        wq_pool = ctx.enter_context(tc.tile_pool(name="wq", bufs=1))
        wqkv_sb = []
        for cc in range(NCC):
            t = wq_pool.tile([P, QKV_CH], F32R, tag=f"wq{cc}")
            nc.sync.dma_start(t[:], wqkv_d[cc * P : (cc + 1) * P, :])
            wqkv_sb.append(t)

        wp_pool = ctx.enter_context(tc.tile_pool(name="wp", bufs=1))
        wpf_a = wp_pool.tile([P, C], F32, tag="wpfa")
        nc.sync.dma_start(wpf_a[:], wproj_d[0:P, :])
        wpf_b = wp_pool.tile([HD, C], F32, tag="wpfb")
        nc.sync.dma_start(wpf_b[:], wproj_d[P : P + HD, :])
        wp_a = wp_pool.tile([P, C], BF16, tag="wpa")
        nc.vector.tensor_copy(out=wp_a[:], in_=wpf_a[:])
        wp_b = wp_pool.tile([HD, C], BF16, tag="wpb")
        nc.vector.tensor_copy(out=wp_b[:], in_=wpf_b[:])

        big_pool = ctx.enter_context(tc.tile_pool(name="big", bufs=1))
        xs_pool = ctx.enter_context(tc.tile_pool(name="xs", bufs=3))
        qkvt_pool = ctx.enter_context(tc.tile_pool(name="qkvt", bufs=1))
        v_pool = ctx.enter_context(tc.tile_pool(name="v", bufs=1))
        y_pool = ctx.enter_context(tc.tile_pool(name="y", bufs=1))
        nrm_pool = ctx.enter_context(tc.tile_pool(name="nrm", bufs=2))
        out_pool = ctx.enter_context(tc.tile_pool(name="outp", bufs=3))

        # ---------------- phase 1: x -> xT (PE transposes, fp32r) ----------
        xT = big_pool.tile([P, NCC * T], F32R, tag="big")  # xT[:, 2048*cc + t]
        with tc.tile_pool(name="ps_xp", bufs=3, space="PSUM") as ps_xp:
            for tt in range(NT):
                xt = xs_pool.tile([P, C], F32R, tag="x")
                nc.sync.dma_start(xt[:], x_d[tt * P : (tt + 1) * P, :])
                for grp, ncc in ((0, 4), (4, 2)):
                    pst = ps_xp.tile([P, ncc * P], F32R, tag="xp")
                    for k in range(ncc):
                        cc = grp + k
                        nc.tensor.transpose(
                            pst[:, k * P : (k + 1) * P],
                            xt[:, cc * P : (cc + 1) * P],
                            ident[:],
                        )
                    for k in range(ncc):
                        cc = grp + k
                        nc.vector.tensor_copy(
                            out=xT[:, cc * T + tt * P : cc * T + (tt + 1) * P],
                            in_=pst[:, k * P : (k + 1) * P],
                        )

        # ---------------- phase 2: qkvT = Wqkv_local.T @ xT (fp32r) --------
        # qkvT partition-tiles: [q0|q1], [k0|k1], [q2|v0], [k2|v1], [v2]
        ch_tiles = [(0, P), (P, P), (2 * P, P), (3 * P, P), (4 * P, HD)]
        qkvT = []
        for i, (ch0, chw) in enumerate(ch_tiles):
            qkvT.append(
                qkvt_pool.tile([chw, T], BF16, tag=f"qkvt{i}", name=f"qkvT{i}")
            )
        with tc.tile_pool(name="ps_qkv", bufs=4, space="PSUM") as ps_qkv:
            for i, (ch0, chw) in enumerate(ch_tiles):
                for tch in range(NQ):
                    ps = ps_qkv.tile([chw, 512], F32, tag="qkv")
                    for cc in range(NCC):
                        nc.tensor.matmul(
                            ps[:],
                            wqkv_sb[cc][:, ch0 : ch0 + chw],
                            xT[:, cc * T + tch * 512 : cc * T + (tch + 1) * 512],
                            start=(cc == 0),
                            stop=(cc == NCC - 1),
                        )
                    nc.vector.tensor_copy(
                        out=qkvT[i][:, tch * 512 : (tch + 1) * 512], in_=ps[:]
                    )

        # head slices (tile index, partition offset)
        q_sl = [(0, 0), (0, HD), (2, 0)]
        k_sl = [(1, 0), (1, HD), (3, 0)]
        v_sl = [(2, HD), (3, HD), (4, 0)]

        # ---------------- phase 2.5: v^T -> v (+ ones col), bf16 -----------
        # v_sb[h]: [128, 16*65]; col 65*jt+64 is the ones column
        v_sb = []
        with tc.tile_pool(name="ps_vt", bufs=3, space="PSUM") as ps_vt:
            for h in range(HPG):
                vt = v_pool.tile([P, NT * (HD + 1)], BF16, tag=f"v{h}")
                ones_cols = vt[:].rearrange("p (t d) -> p t d", d=HD + 1)[:, :, HD:]
                src_ones = ones_f32[:].rearrange("p (a b) -> p a b", b=1)[:, 0:NT, :]
                nc.vector.tensor_copy(out=ones_cols, in_=src_ones)
                ti, po = v_sl[h]
                vTh = qkvT[ti][po : po + HD, :]
                idnb = identb[po : po + HD, po : po + HD]
                for half in range(2):  # 8 t-tiles per psum tile
                    pst = ps_vt.tile([P, 8 * HD], BF16, tag="vt")
                    for k in range(8):
                        jt = half * 8 + k
                        nc.tensor.transpose(
                            pst[:, k * HD : (k + 1) * HD],
                            vTh[:, jt * P : (jt + 1) * P],
                            idnb,
                        )
                    for k in range(8):
                        jt = half * 8 + k
                        nc.vector.tensor_copy(
                            out=vt[:, jt * (HD + 1) : jt * (HD + 1) + HD],
                            in_=pst[:, k * HD : (k + 1) * HD],
                        )
                v_sb.append(vt)

        # ---------------- phase 3: attention ----------------
        # exp buffer: [128, 2*EXPW] bf16; slot 0 = first head of the pair,
        # slot 1 = second. h2 reuses slot 0.
        exp_sb = big_pool.tile([P, 2 * EXPW], BF16, tag="big")
        yT_a = y_pool.tile([P, T], BF16, tag="ya")   # h0 rows 0:64, h1 64:128
        yT_b = y_pool.tile([HD, T], BF16, tag="yb")  # h2

        def ydst_of(h):
            return yT_a[0:HD, :] if h == 0 else (
                yT_a[HD:P, :] if h == 1 else yT_b[0:HD, :]
            )

        with tc.tile_pool(name="ps_att", bufs=1, space="PSUM") as ps_att:
            for pair in ((0, 1), (2,)):
                for j in range(NT):
                    w = T - P * j
                    tq0 = P * j
                    for sl, h in enumerate(pair):
                        qi, qo = q_sl[h]
                        ki, ko = k_sl[h]
                        qh = qkvT[qi][qo : qo + HD, :]
                        kh = qkvT[ki][ko : ko + HD, :]
                        eoff = sl * EXPW + OFFS[j]
                        done = 0
                        while done < w:
                            cw = min(1024, w - done)
                            st = ps_att.tile([P, 1024], F32, tag="st", bufs=3)
                            for s0 in range(0, cw, 512):
                                sw = min(512, cw - s0)
                                nc.tensor.matmul(
                                    st[:, s0 : s0 + sw],
                                    kh[:, tq0 : tq0 + P],
                                    qh[:, tq0 + done + s0 : tq0 + done + s0 + sw],
                                    start=True,
                                    stop=True,
                                )
                            nc.scalar.activation(
                                exp_sb[:, eoff + done : eoff + done + cw],
                                st[:, 0:cw],
                                mybir.ActivationFunctionType.Exp,
                                scale=0.125,
                            )
                            done += cw
                        # causal mask on the diagonal 128-block (GpSimd)
                        dg = exp_sb[:, eoff : eoff + P]
                        nc.vector.tensor_mul(out=dg, in0=dg, in1=tri[:])

                    if j % 4 == 3:
                        q = j // 4
                        for sl, h in enumerate(pair):
                            yq = ps_att.tile([HD + 1, 512], F32, tag="y", bufs=2)
                            for jj in range(4 * q + 4):
                                va = v_sb[h][:, jj * (HD + 1) : (jj + 1) * (HD + 1)]
                                lo = max(512 * q, P * jj)
                                hi = 512 * (q + 1)
                                so = sl * EXPW + OFFS[jj] - P * jj
                                nc.tensor.matmul(
                                    yq[:, lo - 512 * q : hi - 512 * q],
                                    va,
                                    exp_sb[:, so + lo : so + hi],
                                    start=(jj == 0),
                                    stop=(jj == 4 * q + 3),
                                )
                            # normalize chunk: y = y[0:64] * bcast(1/denom)
                            rcp = nrm_pool.tile([P, 512], F32R, tag="rcp")
                            with nc.allow_low_precision(reason="softmax denom"):
                                nc.vector.reciprocal(
                                    rcp[HD : HD + 1, :], yq[HD : HD + 1, :]
                                )
                            bc = ps_att.tile([HD, 512], F32, tag="y", bufs=2)
                            nc.tensor.matmul(
                                bc[:],
                                ones64[HD : HD + 1, :],
                                rcp[HD : HD + 1, :],
                                start=True,
                                stop=True,
                            )
                            bcs = nrm_pool.tile([HD, 512], F32, tag="bcs")
                            nc.vector.tensor_copy(out=bcs[:], in_=bc[:])
                            nc.vector.tensor_mul(
                                out=ydst_of(h)[:, 512 * q : 512 * (q + 1)],
                                in0=yq[0:HD, :],
                                in1=bcs[:],
                            )

        # ---------------- phase 4: proj (bf16, K=128 + K=64) ----------------
        with tc.tile_pool(name="ps_prj", bufs=3, space="PSUM") as ps_prj:
            for tt in range(NT):
                pj = ps_prj.tile([P, C], F32, tag="pj")
                lhs_w = [
                    (yT_a[:, tt * P : (tt + 1) * P], wp_a[:, :]),
                    (yT_b[:, tt * P : (tt + 1) * P], wp_b[:, :]),
                ]
                for ki_, (lhs, wrow) in enumerate(lhs_w):
                    for n0, nw in ((0, 512), (512, 256)):
                        nc.tensor.matmul(
                            pj[:, n0 : n0 + nw],
                            lhs,
                            wrow[:, n0 : n0 + nw],
                            start=(ki_ == 0),
                            stop=(ki_ == 1),
                        )
                ot = out_pool.tile([P, C], F32, tag="o")
                nc.vector.tensor_copy(out=ot[:], in_=pj[:])
                nc.sync.dma_start(out_d[tt * P : (tt + 1) * P, :], ot[:])


@functools.cache
def _build():
    nc = bacc.Bacc(
        "TRN2",
        target_bir_lowering=False,
        debug=False,
        enable_asserts=False,
        num_devices=8,
    )
    x_d = nc.dram_tensor("x", [T, C], F32R, kind="ExternalInput").ap()
    wqkv_d = nc.dram_tensor("wqkv", [C, QKV_CH], F32R, kind="ExternalInput").ap()
    wproj_d = nc.dram_tensor("wproj", [LCH, C], F32, kind="ExternalInput").ap()
    out_d = nc.dram_tensor("out", [T, C], F32, kind="ExternalOutput").ap()
    with tile.TileContext(nc) as tc:
        _emit(nc, tc, x_d, wqkv_d, wproj_d, out_d)
    nc.compile()
    return nc


def kernel(x, mask, Wqkv, Wproj):
    global LAST_RESULT
    x = np.ascontiguousarray(np.asarray(x, dtype=np.float32))
    Wqkv = np.asarray(Wqkv, dtype=np.float32)
    Wproj = np.asarray(Wproj, dtype=np.float32)

    in_maps = []
    for c in range(8):
        b, g = divmod(c, 4)
        hs = [3 * g, 3 * g + 1, 3 * g + 2]  # global heads

        def qcol(h):
            return Wqkv[:, 64 * h : 64 * h + 64]

        def kcol(h):
            return Wqkv[:, C + 64 * h : C + 64 * h + 64]

        def vcol(h):
            return Wqkv[:, 2 * C + 64 * h : 2 * C + 64 * h + 64]

        wq = np.concatenate(
            [
                qcol(hs[0]), qcol(hs[1]),
                kcol(hs[0]), kcol(hs[1]),
                qcol(hs[2]), vcol(hs[0]),
                kcol(hs[2]), vcol(hs[1]),
                vcol(hs[2]),
            ],
            axis=1,
        )
        wp = Wproj[LCH * g : LCH * (g + 1), :]
        in_maps.append(
            {
                "x": np.ascontiguousarray(x[b]),
                "wqkv": np.ascontiguousarray(wq),
                "wproj": np.ascontiguousarray(wp),
            }
        )

    nc = _build()
    res = run_bass_kernel_spmd(nc, in_maps, core_ids=list(range(8)))
    LAST_RESULT = res
    out = np.empty((B, T, C), dtype=np.float32)
    for b in range(B):
        acc = res.results[4 * b]["out"].astype(np.float32)
        for g in range(1, 4):
            acc = acc + res.results[4 * b + g]["out"]
        out[b] = acc
    return out


if __name__ == "__main__":
    rng = np.random.default_rng(0)
    x = rng.standard_normal((B, T, C), dtype=np.float32)
    wqkv = rng.standard_normal((C, 3 * C), dtype=np.float32) / np.sqrt(C)
    wproj = rng.standard_normal((C, C), dtype=np.float32) / np.sqrt(C)
    o = kernel(x, None, wqkv, wproj)
    print(o.shape, o.dtype)
